# revision 40
# baseline (speedup 1.0000x reference)
"""Trainium2 kernel for nn_CP1_17669495456474 (sparse_attention).
8-core data-parallel: core = (sample, spatial half). Device computes the
grouped cross-correlation (per core: 2016 f-positions x 1024 kernels,
K=1024 contraction) as fp16 tensor-engine matmuls. Contraction is split
as partition=(d, c) with dy=2*dy1+d, so the dy1/dx kernel offsets are
free-dim AP shifts and the inputs need only 2x replication (1.7MB/core
total); host applies the cheap fuse/mask/softmax."""
import sys, types
import numpy as np

import concourse.bass as bass
import concourse.mybir as mybir
from concourse.tile import TileContext
import concourse.tile as tile_mod
import concourse.bass_utils as bass_utils

F16 = mybir.dt.float16
F32 = mybir.dt.float32
AOT = mybir.AluOpType
NT, TP, L = 16, 126, 1024

# ---------------- compile workarounds (walrus sync-wait limits) ----------------
import orjson

def _patched_drain_and_barrier(self, tick_clock, wait_clock):
    nc = self.nc
    ScopedClock = tile_mod.ScopedClock
    drain_inst = nc.sync.drain()
    wait_clock.add_sem_waits(drain_inst.ins, ScopedClock({None: tick_clock.global_clock}))
    waits = list(drain_inst.ins.sync_info.on_wait)
    if len(waits) > 1:
        import bass_rust
        drain_inst.ins.sync_info = bass_rust.SyncInfo(on_wait=waits[:1], on_update=[])
        for i in range(1, len(waits)):
            d2 = nc.sync.drain()
            d2.ins.sync_info = bass_rust.SyncInfo(on_wait=[waits[i]], on_update=[])
    nc.all_engine_barrier()
    popped = nc._tile_sem_poison_stack.pop()
    assert popped is self._sem_poison
    nc.clear_and_free_semaphores(list(self.sems.allocated().values()))
    nc.all_engine_barrier()

def _dedup_ldweights(m):
    # Drop Ldweights identical to the weights already resident in the PE
    # array (consecutive same-weight loads, e.g. the n=0/n=1 matmul pairs).
    # Sync carried by a dropped load moves to the next PE instruction.
    for f in m.get("functions", []):
        for b in f.get("blocks", []):
            out = []
            last_sig = None
            carry_w, carry_u = [], []
            for inst in b.get("instructions", []):
                if inst.get("engine") == "PE":
                    opc = inst.get("opcode", "")
                    if opc == "Ldweights":
                        sig = orjson.dumps([inst.get("ins"),
                                            inst.get("tile_position"),
                                            inst.get("tile_size"),
                                            inst.get("perf_mode"),
                                            inst.get("is_transpose")])
                        if sig == last_sig:
                            si = inst.get("sync_info") or {}
                            carry_w.extend(si.get("on_wait") or [])
                            carry_u.extend(si.get("on_update") or [])
                            continue
                        last_sig = sig
                    elif opc != "Matmult":
                        last_sig = None
                    if carry_w or carry_u:
                        si = inst.get("sync_info")
                        if si is None:
                            si = {"on_update": [], "on_wait": []}
                            inst["sync_info"] = si
                        si["on_wait"] = carry_w + (si.get("on_wait") or [])
                        si["on_update"] = (si.get("on_update") or []) + carry_u
                        carry_w, carry_u = [], []
                out.append(inst)
            b["instructions"] = out
    return m

def _split_waits_json(bir_bytes):
    m = _dedup_ldweights(orjson.loads(bir_bytes))
    for f in m.get("functions", []):
        for b in f.get("blocks", []):
            insts = b.get("instructions", [])
            out = []
            for inst in insts:
                si = inst.get("sync_info")
                waits = (si or {}).get("on_wait") or []
                opc = inst.get("opcode", "")
                is_dma = opc.startswith("DMA") or "Trigger" in opc or "Dma" in opc
                keep = 1
                if is_dma and len(waits) <= 1:
                    out.append(inst)
                    continue
                if len(waits) > keep:
                    si["on_wait"] = waits[-keep:]
                    for i, w in enumerate(waits[:-keep]):
                        out.append({
                            "debug": inst.get("debug", 0), "engine": inst["engine"],
                            "ins": [], "outs": [], "name": f"{inst['name']}_xw{i}",
                            "opcode": "EventSemaphore",
                            "sync_info": {"on_update": [], "on_wait": [w]},
                        })
                out.append(inst)
            b["instructions"] = out
    return orjson.dumps(m)

def _install_patches():
    if getattr(bass_utils.compile_bir_kernel, "_wait_split", False):
        return
    TileContext._drain_and_barrier = _patched_drain_and_barrier
    import concourse.bass2jax as b2j
    orig = bass_utils.compile_bir_kernel
    def wrapped(bir_str, *a, **kw):
        if isinstance(bir_str, (bytes, bytearray)):
            try:
                bir_str = _split_waits_json(bir_str)
            except Exception:
                pass
        return orig(bir_str, *a, **kw)
    wrapped._wait_split = True
    bass_utils.compile_bir_kernel = wrapped
    if hasattr(b2j, "compile_bir_kernel"):
        b2j.compile_bir_kernel = wrapped
    # NTFF hook shim so trace=True doesn't crash if requested elsewhere
    if "antenv.axon_hooks" not in sys.modules:
        mod = types.ModuleType("antenv.axon_hooks")
        mod._hook = None
        mod.set_axon_ntff_profile_hook = lambda h: setattr(mod, "_hook", h)
        mod.get_axon_ntff_profile_hook = lambda: mod._hook
        sys.modules["antenv.axon_hooks"] = mod
        try:
            from trn_agent_boot.trn_boot import _ntff_profile_via_ctypes
            hk = _ntff_profile_via_ctypes('/opt/axon/libaxon_pjrt.so')
            if hk is not None:
                mod._hook = hk
        except Exception:
            pass
        bass_utils.upload_artifacts = lambda tmpdir: str(tmpdir)

# ---------------- device program: raw cos in [p, l] tiles ----------------
_NC_CACHE = [None]

def _build_nc():
    if _NC_CACHE[0] is not None:
        return _NC_CACHE[0]
    _install_patches()
    nc = bass.Bass("TRN2", target_bir_lowering=False, debug=False)
    g0_d = nc.dram_tensor("g0", [128, 4, 6, 63], F16, kind="ExternalInput")
    g1_d = nc.dram_tensor("g1", [128, 4, 18, 63], F16, kind="ExternalInput")
    g2_d = nc.dram_tensor("g2", [128, 4, 14, 63], F16, kind="ExternalInput")
    K_d = nc.dram_tensor("kn", [128, 65, 66], F16, kind="ExternalInput")
    o_d = nc.dram_tensor("o", [NT, TP, L], F16, kind="ExternalOutput")
    with TileContext(nc) as tc:
        import contextlib
        ctx = contextlib.ExitStack()
        with ctx:
            const = ctx.enter_context(tc.tile_pool(name="const", bufs=1))
            outp = ctx.enter_context(tc.tile_pool(name="outp", bufs=3))
            psp = ctx.enter_context(tc.tile_pool(name="psp", bufs=3, space="PSUM"))
            psw = ctx.enter_context(tc.tile_pool(name="psw", bufs=1, space="PSUM"))
            Gh0 = const.tile([128, 4, 6, 63], F16, tag="Gh0")
            Gh1 = const.tile([128, 4, 18, 63], F16, tag="Gh1")
            Gh2 = const.tile([128, 4, 14, 63], F16, tag="Gh2")
            Ks = const.tile([128, 65, 66], F16, tag="Ks")
            Wd = const.tile([128, 126], F16, tag="Wd")
            Xd = const.tile([128, 512], F16, tag="Xd")
            # warmup: ramp the PE p-state while the inputs land
            nc.gpsimd.memset(Wd[:], 0.0)
            nc.gpsimd.memset(Xd[:], 0.0)
            pw = psw.tile([128, 512], F32, tag="pw")
            for _ in range(24):
                nc.tensor.matmul(pw[0:TP, :], Wd[:], Xd[:], start=True, stop=True,
                                 skip_group_check=True)
            # Inputs are near-compact (Ks 2x-replicated only, G split into 3
            # y-chunks). Wave 1 = everything tile 0/1 needs (Ks + g0), spread
            # finely across queues; g1/g2 follow and land well before use.
            engs = [nc.scalar, nc.sync, nc.gpsimd]
            q = 0
            # tile 0's first half needs Ks y<40 and all of Gh0 -> issue those
            # interleaved so every engine's earliest triggers cover them
            wave1 = []
            for y0, y1 in ((0, 8), (8, 16), (16, 24), (24, 32), (32, 40)):
                wave1.append((Ks[:, y0:y1], K_d[:, y0:y1]))
            for p0 in range(0, 128, 32):
                wave1.append((Gh0[p0:p0+32], g0_d[p0:p0+32]))
            for y0, y1 in ((40, 48), (48, 56), (56, 65)):
                wave1.append((Ks[:, y0:y1], K_d[:, y0:y1]))
            for dst, src in wave1:
                engs[q % 3].dma_start(out=dst, in_=src)
                q += 1
            for dx in range(4):
                engs[q % 3].dma_start(out=Gh1[:, dx], in_=g1_d[:, dx])
                q += 1
            for dx in range(4):
                engs[q % 3].dma_start(out=Gh2[:, dx], in_=g2_d[:, dx])
                q += 1

            for t in range(NT):
                ps = psp.tile([128, 1024], F32, tag="ps", name="ps")
                if t < 2:
                    Gc, ly = Gh0, 2*t
                elif t < 10:
                    Gc, ly = Gh1, 2*t - 4
                else:
                    Gc, ly = Gh2, 2*t - 20
                if t == 0 or t == NT - 1:
                    # n-first. Tile 0: the first 8 matmuls need only Ks rows
                    # 0:40, which land earlier (y-sliced load pieces). Last
                    # tile: the n=0 PSUM half finishes at matmul 8, letting
                    # its copy+DMA overlap the final 8 matmuls.
                    order = [(dy1, dx, n) for n in range(2) for dy1 in range(2)
                             for dx in range(4)]
                    starts = {0, 8}
                    stops = {7, 15}
                else:
                    order = [(dy1, dx, n) for dy1 in range(2) for dx in range(4)
                             for n in range(2)]
                    starts = {0, 1}
                    stops = {14, 15}
                for kk, (dy1, dx, n) in enumerate(order):
                    y0 = ly + 2*dy1
                    lhsT = Gc[:, dx, y0:y0+2, :]
                    rhs = Ks[:, 32*n+2*dy1:32*n+2*dy1+31:2, dx:dx+63:2]
                    nc.tensor.matmul(ps[0:TP, 512*n:512*n+512], lhsT, rhs,
                                     start=(kk in starts), stop=(kk in stops),
                                     skip_group_check=True)
                O = outp.tile([128, 1024], F16, tag="O", name="O")
                nc.scalar.copy(out=O[0:TP, 0:512], in_=ps[0:TP, 0:512])
                nc.vector.tensor_copy(O[0:TP, 512:1024], ps[0:TP, 512:1024])
                if t == NT - 1:
                    # keep gpsimd out of the final transfers so its (slowest)
                    # teardown drain starts as early as possible
                    e0, e1 = nc.sync, nc.scalar
                else:
                    e0, e1 = (nc.sync, nc.gpsimd) if t % 2 == 0 else (nc.gpsimd, nc.sync)
                e0.dma_start(out=o_d[t, :, 0:512], in_=O[0:TP, 0:512])
                e1.dma_start(out=o_d[t, :, 512:1024], in_=O[0:TP, 512:1024])
    _NC_CACHE[0] = nc
    return nc

# ---------------- host side ----------------
def _pad_edge3(x):
    return np.pad(x, ((0, 0), (1, 1), (1, 1)), mode='edge')

def _build_K(bnpad16):
    # [128, 65, 66]: K[(d,c), y, x] = bnpad[c, y+d, x]  (d = dy & 1)
    K = np.empty((128, 65, 66), np.float16)
    for d in range(2):
        K[64*d:64*d+64] = bnpad16[:, d:d+65, :]
    return K

def _build_G(fpad16, half):
    # [128, 4, 34, 63]: G[(d,c), dx, y, x] = fpad[c, r0+y+d, dx+x]
    r0 = 0 if half == 0 else 31
    G = np.empty((128, 4, 34, 63), np.float16)
    for d in range(2):
        for dx in range(4):
            G[64*d:64*d+64, dx] = fpad16[:, r0+d:r0+d+34, dx:dx+63]
    return G

def _make_in_maps(f, b):
    f = np.asarray(f, dtype=np.float32)
    b = np.asarray(b, dtype=np.float32)
    in_maps = []
    for smp in range(4):
        bs = b[smp]
        bn = bs / np.sqrt((bs*bs).sum(axis=(1, 2), keepdims=True) + 1e-8)
        Kmat = _build_K(_pad_edge3(bn).astype(np.float16))
        fpad16 = _pad_edge3(f[smp]).astype(np.float16)
        for half in range(2):
            G = _build_G(fpad16, half)
            in_maps.append({
                "g0": np.ascontiguousarray(G[:, :, 0:6]),
                "g1": np.ascontiguousarray(G[:, :, 4:22]),
                "g2": np.ascontiguousarray(G[:, :, 20:34]),
                "kn": Kmat,
            })
    return in_maps

def _diag3(x):
    out = x.copy()
    out[:, :, 1:, 1:] += x[:, :, :-1, :-1]
    out[:, :, :-1, :-1] += x[:, :, 1:, 1:]
    return out

def _host_post_full(cos, maskc):
    # cos (B,1024,63,63) fp32, maskc (B,1,64,64) -> softmax out (B,1024,63,63)
    B = cos.shape[0]
    cs, hs, ws = 1024, 63, 63
    hb = wb = 32
    c1 = _diag3(cos.reshape(B, 1, cs, hs*ws))
    c1 = c1.reshape(B, 1, hb, wb, hs, ws).transpose(0, 1, 3, 2, 5, 4).reshape(B, 1, cs, hs*ws)
    c1 = _diag3(np.ascontiguousarray(c1))
    c1 = c1.reshape(B, 1, hb, wb, hs, ws).transpose(0, 1, 3, 2, 5, 4)
    cos = c1.reshape(B, cs, hs, ws)

    # window sums of maskc via integral image (integer-exact in float64)
    mc = np.pad(maskc[:, 0], ((0, 0), (1, 1), (1, 1)), mode='edge').astype(np.float64)
    I = np.zeros((B, 67, 67), np.float64)
    I[:, 1:, 1:] = mc.cumsum(axis=1).cumsum(axis=2)
    S = I[:, 4:, 4:] - I[:, :-4, 4:] - I[:, 4:, :-4] + I[:, :-4, :-4]  # (B,63,63)
    Sp = S                              # stride-1 windows
    Sk = S[:, ::2, ::2]                 # stride-2 windows (32x32)
    mm = (Sk[:, :, :, None, None].reshape(B, cs, 1, 1) > Sp[:, None, :, :])
    ppp = (Sp > 8.0)[:, None, :, :]
    mm = mm & ppp | (Sk.reshape(B, cs, 1, 1) == 16.0)
    cos = cos * mm.astype(np.float32)

    z = cos * 10.0
    z -= z.max(axis=1, keepdims=True)
    np.exp(z, out=z)
    z /= z.sum(axis=1, keepdims=True)
    return z

def kernel(f, b, mask):
    f = np.asarray(f, dtype=np.float32)
    b = np.asarray(b, dtype=np.float32)
    mask = np.asarray(mask, dtype=np.float32)
    B = f.shape[0]
    maskc = 1.0 - mask
    nc = _build_nc()
    in_maps = _make_in_maps(f, b)
    res = bass_utils.run_bass_kernel_spmd(nc, in_maps, list(range(8)))
    cos = np.empty((B, L, 63, 63), np.float32)
    for core in range(8):
        smp, half = core // 2, core % 2
        r0 = 0 if half == 0 else 31
        a = np.asarray(res.results[core]["o"], dtype=np.float32)
        h = a.reshape(NT, 2, 63, L).transpose(3, 0, 1, 2).reshape(L, 32, 63)
        cos[smp, :, r0:r0+32, :] = h
    return _host_post_full(cos, maskc)
